# revision 15
# baseline (speedup 1.0000x reference)
"""Trainium2 Bass kernel for the conditioned WaveNet denoiser.

Distribution strategy (8 NeuronCores):
  - Data-parallel over batch: core b owns sample b end-to-end (block loop +
    output head), with the small weights replicated.
  - The huge stacked conditioning Dense weights Dt/Ds ([10,16,2048,128] f32,
    ~335 MB for the pair) are channel-sharded 8 ways.  Core j computes the
    conditioning planes trans[b, t, k] for ALL batches b over its 16 channels
    (a packed bf16 matmul against a host-built block-diagonal matrix of
    `condition`), then a chunked AllToAll routes each batch's planes to its
    owner core, overlapped with the residual-block compute.

v3 changes (trace-driven):
  - A tiny dummy AllToAll issues at t~0 to absorb the ~50us collective entry
    barrier + launch skew, which otherwise delays the first real AllToAll.
  - Conditioning planes travel int8 on the wire (per-(l,branch) scale
    computed host-side; 1/s folded into the dilated-conv weights and s
    restored via the activation's scale operand) -> AllToAll bytes and HBM
    bounce traffic halve; the serialized AllToAll chain shrinks ~2x.
  - Producer chunks and consumer (residual-block) chunks interleave on the
    in-order PE stream: P0 P1 C0 P2 C1 ... C4.
  - skip_sum accumulates in 4 persistent PSUM banks (one per time tile)
    across all 10 blocks -> no per-block DVE skip add.
  - Stationary weights are reused across the 4 time tiles (tap-outer loops)
    in both the block convs and the output head -> ~4x fewer LDWEIGHTS.
  - Producer PSUM->SBUF quantize-copies and h_bf casts run on the scalar
    engine; gate activations emit bf16 so the gate multiply runs in DVE 2x
    mode.  Plane tiles fetch on the vector engine's DMA queue; output-head
    weights prefetch at kernel start on the scalar queue.

kernel() accepts the FULL inputs and returns the FULL [8, 2048, 1] output.
"""

import os
import sys

import numpy as np

for _p in ("/opt/trn_rl_repo",):
    if _p not in sys.path and os.path.isdir(_p):
        sys.path.insert(0, _p)

import ml_dtypes  # noqa: E402

import concourse.bass as bass  # noqa: E402
import concourse.tile as tile  # noqa: E402
from concourse import bacc, bass_utils, mybir  # noqa: E402

# Problem constants (hardcoded per the spec; kernel.py must be self-contained).
L = 10
DILATIONS = [1, 2, 4, 8, 16, 32, 64, 128, 256, 512]
T = 2048
C = 128
COND = 16
B = 8
NCORES = 8
TS = 512          # time-tile (matmul moving N / one PSUM bank of f32)
NT = T // TS      # 4 time tiles
CHUNK = 2         # residual blocks per AllToAll chunk
NCHUNK = L // CHUNK
PLANES_PER_CHUNK = 2 * CHUNK  # (l, branch) planes per chunk

F32 = mybir.dt.float32
F32R = mybir.dt.float32r
BF16 = mybir.dt.bfloat16
I8 = mybir.dt.int8
BF = ml_dtypes.bfloat16

AF = mybir.ActivationFunctionType


def _tap_range(t0, n, off):
    """Valid (out_lo, length) of an out tile [t0, t0+n) for input offset off."""
    lo = max(t0, -off)
    hi = min(t0 + n, T - off)
    return lo - t0, max(0, hi - lo)


def _conv_taps(offsets):
    """[(tap, off, [(it, lo, n), ...]), ...] for valid pieces, center first."""
    taps = []
    for tap, off in offsets:
        pieces = []
        for it in range(NT):
            lo, n = _tap_range(it * TS, TS, off)
            if n > 0:
                pieces.append((it, lo, n))
        if pieces:
            taps.append((tap, off, pieces))
    return taps


MAGIC = 12582912.0  # 1.5 * 2**23: f32 add/sub rounds to nearest integer


def _build_nc(has_p: bool, has_bres: bool, has_bskip: bool):
    """Planes ship as int8 tt/s with per-(l,branch,batch) scales supplied as
    input tensors: conv weights come pre-multiplied by 1/s (per-core) and the
    gate activation applies scale=s via a per-partition scale AP.  The
    quantize step rounds explicitly via the f32 magic-number trick so the
    result is exact under any convert-rounding mode."""
    nc = bacc.Bacc(
        "TRN2",
        target_bir_lowering=False,
        debug=False,
        num_devices=NCORES,
    )

    # ---- I/O declarations (per-core values supplied via in_maps) ----
    xw = nc.dram_tensor("xw", [1, T], F32, kind="ExternalInput")
    wcT = nc.dram_tensor("wcT", [1, C], F32, kind="ExternalInput")
    bcp = nc.dram_tensor("bcp", [C, 1], F32, kind="ExternalInput")
    cstat = nc.dram_tensor("cstat", [C, 64], BF16, kind="ExternalInput")
    # [lb, p, hh, t] so one plane-pair is a single contiguous-line DMA
    dtp = nc.dram_tensor("dtp", [2 * L, C, 2, T], BF16, kind="ExternalInput")
    wtp = nc.dram_tensor("wtp", [C, 6 * L, C], BF16, kind="ExternalInput")
    wsr = nc.dram_tensor("wsr", [C, 2 * L, C], BF16, kind="ExternalInput")
    w1p = nc.dram_tensor("w1p", [C, 3, 2048], BF16, kind="ExternalInput")
    b1p = nc.dram_tensor("b1p", [C, 16], F32, kind="ExternalInput")
    w2p = nc.dram_tensor("w2p", [C, 96, C], BF16, kind="ExternalInput")
    b2p = nc.dram_tensor("b2p", [C, 2], F32, kind="ExternalInput")
    w3p = nc.dram_tensor("w3p", [C, 2], BF16, kind="ExternalInput")
    b3p = nc.dram_tensor("b3p", [1, 1], F32, kind="ExternalInput")
    invsp = nc.dram_tensor("invsp", [C, 2 * L], F32, kind="ExternalInput")
    spp = nc.dram_tensor("spp", [C, 2 * L], F32, kind="ExternalInput")
    if has_p:
        ptp = nc.dram_tensor("ptp", [2 * L, 16, T], BF16, kind="ExternalInput")
        pstat = nc.dram_tensor("pstat", [8, 64], BF16, kind="ExternalInput")
    if has_bres:
        bresp = nc.dram_tensor("bresp", [C, L], F32, kind="ExternalInput")
    if has_bskip:
        bskips = nc.dram_tensor("bskips", [C, 1], F32, kind="ExternalInput")
    out = nc.dram_tensor("out", [1, T], F32, kind="ExternalOutput")

    rg = [list(range(NCORES))]

    with tile.TileContext(nc) as tc:
        with (
            tc.tile_pool(name="consts", bufs=1) as consts,
            tc.tile_pool(name="skipbuf", bufs=1) as skipbuf,
            tc.tile_pool(name="headw", bufs=1) as headw,
            tc.tile_pool(name="dram", bufs=1, space="DRAM") as dram,
        ):
            # ---- small constants (hot path first; sync queue) ----
            x_sb = consts.tile([1, T], F32)
            nc.sync.dma_start(x_sb[:], xw[:, :])
            wc_sb = consts.tile([1, C], F32)
            nc.sync.dma_start(wc_sb[:], wcT[:, :])
            bc_sb = consts.tile([C, 1], F32)
            nc.sync.dma_start(bc_sb[:], bcp[:, :])
            cs_sb = consts.tile([C, 64], BF16)
            nc.sync.dma_start(cs_sb[:], cstat[:, :])
            b1_sb = headw.tile([C, 16], F32)
            nc.sync.dma_start(b1_sb[:], b1p[:, :])
            b2_sb = headw.tile([C, 2], F32)
            nc.sync.dma_start(b2_sb[:], b2p[:, :])
            w3_sb = headw.tile([C, 2], BF16)
            nc.sync.dma_start(w3_sb[:], w3p[:, :])
            b3_sb = headw.tile([1, 1], F32)
            nc.sync.dma_start(b3_sb[:], b3p[:, :])
            invs_sb = consts.tile([C, 2 * L], F32)
            nc.sync.dma_start(invs_sb[:], invsp[:, :])
            sp_sb = consts.tile([C, 2 * L], F32)
            nc.sync.dma_start(sp_sb[:], spp[:, :])
            if has_p:
                ps_sb = consts.tile([8, 64], BF16)
                nc.sync.dma_start(ps_sb[:], pstat[:, :])
            if has_bres:
                bres_sb = consts.tile([C, L], F32)
                nc.sync.dma_start(bres_sb[:], bresp[:, :])
            if has_bskip:
                bsk_sb = consts.tile([C, 1], F32)
                nc.sync.dma_start(bsk_sb[:], bskips[:, :])

            # ---- block + head weights on the scalar queue (idle early) ----
            wt_sb = consts.tile([C, 6 * L, C], BF16)
            nc.scalar.dma_start(wt_sb[:], wtp[:, :, :])
            wsr_sb = consts.tile([C, 2 * L, C], BF16)
            nc.scalar.dma_start(wsr_sb[:], wsr[:, :, :])
            w1_sb = headw.tile([C, 3, 2048], BF16)
            nc.scalar.dma_start(w1_sb[:], w1p[:, :, :])
            w2_sb = headw.tile([C, 96, C], BF16)
            nc.scalar.dma_start(w2_sb[:], w2p[:, :, :])

            # AllToAll bounce buffers (int8), one pair per chunk.
            a2a_in = []
            a2a_out = []
            for c in range(NCHUNK):
                ain = dram.tile(
                    [B, PLANES_PER_CHUNK, 16, T], I8, name=f"a2a_in{c}"
                )
                aout = dram.tile(
                    [B, PLANES_PER_CHUNK, 16, T], I8, name=f"a2a_out{c}"
                )
                a2a_in.append(ain)
                a2a_out.append(aout)

            # All producer + consumer pools share one scope so PSUM banks and
            # SBUF regions never alias between the two concurrent phases.
            # PSUM budget (8 banks): zA + zB (2 banks each) for the gated
            # convs / residual, and one [C,T] 4-bank region shared temporally
            # by the producer ppr generations and the persistent skip
            # accumulator.
            with (
                tc.tile_pool(name="hbuf", bufs=2) as hbuf,
                tc.tile_pool(name="hbfbuf", bufs=2) as hbfbuf,
                tc.tile_pool(name="dtbuf", bufs=2) as dtbuf,
                tc.tile_pool(name="ptbuf", bufs=2) as ptbuf,
                tc.tile_pool(name="stgbuf", bufs=4) as stgbuf,
                tc.tile_pool(name="stgf", bufs=1) as stgf,
                tc.tile_pool(name="ttbuf", bufs=2) as ttbuf,
                tc.tile_pool(name="gbuf", bufs=2) as gbuf,
                tc.tile_pool(name="zsbuf", bufs=2) as zsbuf,
                tc.tile_pool(name="avbuf", bufs=1) as avbuf,
                tc.tile_pool(name="psum_z", bufs=1, space="PSUM") as psum_z,
                tc.tile_pool(name="psum_skip", bufs=1, space="PSUM") as psum_skip,
            ):
                TS2 = 2 * TS

                # ---- h = x * Wc + bc  (K=1 f32 matmul + biased copy) ----
                h = hbuf.tile([C, T], F32, name="h0")
                h_bf = hbfbuf.tile([C, T], BF16, name="hbf0")
                for half in range(2):
                    ph = psum_z.tile(
                        [C, TS2], F32, name="ph", tag="zA" if half == 0 else "zB"
                    )
                    for sub in range(2):
                        it = 2 * half + sub
                        nc.tensor.matmul(
                            ph[:, bass.ts(sub, TS)],
                            wc_sb[:, :],
                            x_sb[:, bass.ts(it, TS)],
                            start=True,
                            stop=True,
                        )
                    hsl = bass.ts(half, TS2)
                    nc.scalar.activation(
                        h[:, hsl], ph[:], AF.Identity, bias=bc_sb[:, 0:1]
                    )
                    nc.vector.tensor_copy(h_bf[:, hsl], h[:, hsl])

                # ---- producer: conditioning planes for one chunk ----
                def produce_chunk(cki):
                    for lbc in range(PLANES_PER_CHUNK):
                        lb = cki * PLANES_PER_CHUNK + lbc
                        dt2 = dtbuf.tile([C, 2, T], BF16, name="dt2")
                        nc.sync.dma_start(dt2[:], dtp[lb])
                        if has_p:
                            pt = ptbuf.tile([16, T], BF16, name="pt")
                            nc.sync.dma_start(pt[:], ptp[lb])
                        stg = stgbuf.tile([C, T], I8, name="stg")
                        ppr = psum_skip.tile([C, T], F32, name="ppr", tag="skip")
                        for it in range(NT):
                            tsl = bass.ts(it, TS)
                            for hh in range(2):
                                rows = slice(64 * hh, 64 * hh + 64)
                                nc.tensor.matmul(
                                    ppr[rows, tsl],
                                    cs_sb[:, :],
                                    dt2[:, hh, tsl],
                                    start=True,
                                    stop=not has_p,
                                )
                                if has_p:
                                    nc.tensor.matmul(
                                        ppr[rows, tsl],
                                        ps_sb[:, :],
                                        pt[8 * hh : 8 * hh + 8, tsl],
                                        start=False,
                                        stop=True,
                                    )
                        # int8 quantize with explicit RNE rounding:
                        # t = ppr/s + MAGIC (f32 add rounds mantissa),
                        # stg = int8(t - MAGIC) (exact integer convert)
                        tq = stgf.tile([C, T], F32, name="tq")
                        nc.vector.tensor_scalar(
                            tq[:], ppr[:], invs_sb[:, lb : lb + 1], MAGIC,
                            mybir.AluOpType.mult, mybir.AluOpType.add,
                        )
                        nc.scalar.activation(
                            stg[:], tq[:], AF.Copy, bias=-MAGIC
                        )
                        for hh in range(2):
                            nc.scalar.dma_start(
                                a2a_in[cki][:, lbc, 8 * hh : 8 * hh + 8, :],
                                stg[64 * hh : 64 * hh + 64, :],
                            )
                    nc.gpsimd.collective_compute(
                        "AllToAll",
                        mybir.AluOpType.bypass,
                        replica_groups=rg,
                        ins=[a2a_in[cki][:, :, :, :].opt()],
                        outs=[a2a_out[cki][:, :, :, :].opt()],
                    )

                # ---- consumer: load one chunk's planes.  On the sync queue
                # (idle once the dtp stream drains): the HWDGE stalls on the
                # AllToAll-completion semaphore harmlessly, and the gpsimd
                # queue stays pure collectives so the chain has no gaps. ----
                def load_chunk(cki):
                    tb = ttbuf.tile([C, PLANES_PER_CHUNK, T], I8, name="tb")
                    for p in range(PLANES_PER_CHUNK):
                        nc.sync.dma_start(
                            tb[:, p, :], a2a_out[cki][:, p, :, :]
                        )
                    return tb

                # ---- consumer: one residual block ----
                def consume_block(l, tb, psk):
                    nonlocal h, h_bf
                    d = DILATIONS[l]
                    lrel = l % CHUNK
                    planes = [tb[:, 2 * lrel + br, :] for br in range(2)]
                    g = gbuf.tile([C, T], BF16, name="g")
                    h_new = hbuf.tile([C, T], F32, name="hn")
                    h_bf_new = hbfbuf.tile([C, T], BF16, name="hbn")
                    taps = _conv_taps(((1, 0), (0, -d), (2, d)))
                    nvalid = [0] * NT
                    for _, _, pieces in taps:
                        for it, _, _ in pieces:
                            nvalid[it] += 1
                    avs = []
                    for br, fn in ((0, AF.Tanh), (1, AF.Sigmoid)):
                        pz = [
                            psum_z.tile([C, TS2], F32, name=f"pz{br}", tag="zA"),
                            psum_z.tile([C, TS2], F32, name=f"pz{br}b", tag="zB"),
                        ]
                        done = [0] * NT
                        for tap, off, pieces in taps:
                            w_ap = wt_sb[:, (l * 2 + br) * 3 + tap, :]
                            for it, lo, n in pieces:
                                t0 = it * TS
                                col = (it % 2) * TS + lo
                                done[it] += 1
                                nc.tensor.matmul(
                                    pz[it // 2][:, col : col + n],
                                    w_ap,
                                    h_bf[:, t0 + lo + off : t0 + lo + off + n],
                                    start=done[it] == 1,
                                    stop=done[it] == nvalid[it],
                                )
                        av = avbuf.tile([C, T], BF16, name="av", tag=f"av{br}")
                        lb = l * 2 + br
                        for half in range(2):
                            hsl = bass.ts(half, TS2)
                            zs = zsbuf.tile(
                                [C, TS2], F32, name="zs", tag=f"zs{br}"
                            )
                            nc.vector.tensor_add(
                                zs[:], pz[half][:], planes[br][:, hsl]
                            )
                            nc.scalar.activation(
                                av[:, hsl], zs[:], fn,
                                scale=sp_sb[:, lb : lb + 1],
                            )
                        avs.append(av)
                    nc.vector.tensor_mul(g[:, :], avs[0][:, :], avs[1][:, :])
                    # skip 1x1 conv, accumulated in PSUM across all blocks
                    for it in range(NT):
                        nc.tensor.matmul(
                            psk[:, bass.ts(it, TS)],
                            wsr_sb[:, 2 * l, :],
                            g[:, bass.ts(it, TS)],
                            start=l == 0,
                            stop=l == L - 1,
                        )
                    # residual 1x1 conv + h (zA/zB banks, next generation)
                    for half in range(2):
                        hsl = bass.ts(half, TS2)
                        prs = psum_z.tile(
                            [C, TS2], F32, name="prs",
                            tag="zA" if half == 0 else "zB",
                        )
                        for sub in range(2):
                            it = 2 * half + sub
                            nc.tensor.matmul(
                                prs[:, bass.ts(sub, TS)],
                                wsr_sb[:, 2 * l + 1, :],
                                g[:, bass.ts(it, TS)],
                                start=True,
                                stop=True,
                            )
                        nc.vector.tensor_add(h_new[:, hsl], prs[:], h[:, hsl])
                        if has_bres:
                            nc.scalar.activation(
                                h_new[:, hsl],
                                h_new[:, hsl],
                                AF.Identity,
                                bias=bres_sb[:, l : l + 1],
                            )
                        nc.scalar.copy(h_bf_new[:, hsl], h_new[:, hsl])
                    h = h_new
                    h_bf = h_bf_new

                # ---- schedule: produce everything first (DMA-paced), then
                # plane loads (sync queue, gated by each AllToAll), then all
                # blocks.  The gpsimd queue is pure collectives -> the chain
                # runs back-to-back and the PE stays busy (and warm). ----
                for cki in range(NCHUNK):
                    produce_chunk(cki)
                tbs = [load_chunk(cki) for cki in range(NCHUNK)]
                # Persistent skip accumulator: next generation of the shared
                # 4-bank region (chains after the last producer read).
                psk = psum_skip.tile([C, T], F32, name="psk", tag="skip")
                for l in range(L):
                    consume_block(l, tbs[l // CHUNK], psk)

                # ---- skip_sum out of PSUM (bias + bf16 cast on scalar) ----
                skip_bf = skipbuf.tile([C, T], BF16, name="skipbf")
                for half in range(2):
                    hsl = bass.ts(half, TS2)
                    if has_bskip:
                        nc.scalar.activation(
                            skip_bf[:, hsl], psk[:, hsl],
                            AF.Identity, bias=bsk_sb[:, 0:1],
                        )
                    else:
                        nc.scalar.copy(skip_bf[:, hsl], psk[:, hsl])

            # ---- output head ----
            with (
                tc.tile_pool(name="o1buf", bufs=1) as o1buf,
                tc.tile_pool(name="o2buf", bufs=1) as o2buf,
                tc.tile_pool(name="obuf", bufs=1) as obuf,
                tc.tile_pool(name="psum_h1", bufs=1, space="PSUM") as psum_h1,
                tc.tile_pool(name="psum_h2", bufs=1, space="PSUM") as psum_h2,
            ):
                out1 = o1buf.tile([C, 16, T], BF16, name="out1")
                out2 = o2buf.tile([C, 2, T], BF16, name="out2")
                o_sb = obuf.tile([1, T], F32, name="o_sb")
                htaps = _conv_taps(((1, 0), (0, -1), (2, 1)))
                hnv = [0] * NT
                for _, _, pieces in htaps:
                    for it, _, _ in pieces:
                        hnv[it] += 1
                for oc in range(16):
                    p1 = [
                        psum_h1.tile([C, TS], F32, name="p1", tag=f"h1{it}")
                        for it in range(NT)
                    ]
                    done = [0] * NT
                    for tap, off, pieces in htaps:
                        w_ap = w1_sb[:, tap, oc * C : (oc + 1) * C]
                        for it, lo, n in pieces:
                            t0 = it * TS
                            done[it] += 1
                            nc.tensor.matmul(
                                p1[it][:, lo : lo + n],
                                w_ap,
                                skip_bf[:, t0 + lo + off : t0 + lo + off + n],
                                start=done[it] == 1,
                                stop=done[it] == hnv[it],
                            )
                    for it in range(NT):
                        nc.scalar.activation(
                            out1[:, oc, bass.ts(it, TS)],
                            p1[it][:],
                            AF.Relu,
                            bias=b1_sb[:, oc : oc + 1],
                        )
                for oc2 in range(2):
                    p2 = [
                        psum_h2.tile([C, TS], F32, name="p2", tag=f"h2{it}")
                        for it in range(NT)
                    ]
                    done = [0] * NT
                    for tap, off, pieces in htaps:
                        for ic in range(16):
                            w_ap = w2_sb[:, (tap * 16 + ic) * 2 + oc2, :]
                            for it, lo, n in pieces:
                                t0 = it * TS
                                done[it] += 1
                                nc.tensor.matmul(
                                    p2[it][:, lo : lo + n],
                                    w_ap,
                                    out1[:, ic, t0 + lo + off : t0 + lo + off + n],
                                    start=done[it] == 1,
                                    stop=done[it] == 16 * hnv[it],
                                )
                    for it in range(NT):
                        nc.scalar.activation(
                            out2[:, oc2, bass.ts(it, TS)],
                            p2[it][:],
                            AF.Relu,
                            bias=b2_sb[:, oc2 : oc2 + 1],
                        )
                for it in range(NT):
                    tsl = bass.ts(it, TS)
                    p3 = psum_h1.tile([1, TS], F32, name="p3", tag="h10")
                    for ic in range(2):
                        nc.tensor.matmul(
                            p3[:],
                            w3_sb[:, ic : ic + 1],
                            out2[:, ic, tsl],
                            start=ic == 0,
                            stop=ic == 1,
                        )
                    nc.scalar.activation(
                        o_sb[:, tsl], p3[:], AF.Tanh, bias=b3_sb[:, 0:1]
                    )
                nc.sync.dma_start(out[:, :], o_sb[:])

    nc.compile()
    return nc


_NC_CACHE = {}


def _get_nc(has_p, has_bres, has_bskip):
    key = (has_p, has_bres, has_bskip)
    if key not in _NC_CACHE:
        _NC_CACHE[key] = _build_nc(has_p, has_bres, has_bskip)
    return _NC_CACHE[key]


def _pack_inputs(
    x, condition, Wc, bc, Wt, bt, Ws, bs, Dt, Bt, Ds, Bs,
    Wskip, bskip, Wres, bres, W1, b1, W2, b2, W3, b3,
):
    """Host-side sharding + layout packs. Returns (in_maps, flags).

    flags = (has_p, has_bres, has_bskip) — everything _get_nc needs.
    """
    f32 = np.float32
    x = np.asarray(x, f32)
    condition = np.asarray(condition, f32)
    has_p = bool(
        np.any(np.asarray(Bt)) or np.any(np.asarray(Bs))
        or np.any(np.asarray(bt)) or np.any(np.asarray(bs))
    )
    has_bres = bool(np.any(np.asarray(bres)))
    has_bskip = bool(np.any(np.asarray(bskip)))

    # int8 calibration: per-(l,branch,batch) scale from the exact plane
    # amax (margin covers device bf16 rounding).  Scales are runtime data:
    # invsp feeds the producer quantize (partition-encoded by batch), spp
    # feeds each core's gate activation, and 1/s folds into that core's
    # dilated-conv weights.
    Dt_ = np.asarray(Dt, f32)
    Ds_ = np.asarray(Ds, f32)
    scl = np.zeros((L, 2, B), f32)  # s[l, br, b]
    for l in range(L):
        for br, Dn in ((0, Dt_), (1, Ds_)):
            M = condition @ Dn[l].reshape(COND, T * C)  # [B, T*C]
            amax = np.abs(M).max(axis=1)  # per-batch
            if has_p:
                Pn = (np.asarray(Bt if br == 0 else Bs, f32)[l]
                      + np.asarray(bt if br == 0 else bs, f32)[l][None, :])
                amax = amax + float(np.abs(Pn).max())
            scl[l, br] = np.maximum(amax, 1e-6) * 1.05 / 127.0
    # invsp[p=64hh+8b+g, lb] = 1/s[lb, b]  (same on every core)
    invsp = np.zeros((C, 2 * L), f32)
    for hh in range(2):
        for b in range(B):
            for g in range(8):
                invsp[64 * hh + 8 * b + g, :] = (
                    1.0 / scl[:, :, b].reshape(2 * L)
                )
    # spp per core b: s[lb, b] broadcast over partitions
    spp_all = np.ascontiguousarray(
        np.broadcast_to(
            scl.reshape(2 * L, B).T[:, None, :], (B, C, 2 * L)
        ).copy()
    )

    # dtp: [core, lb=2l+br, hh, p=16g+c, t] = D_br[l, c, t, 16j+8hh+g]
    D = np.stack([Dt_, Ds_], axis=1)
    D = D.reshape(L, 2, COND, T, 8, 2, 8)
    # [core, lb, p=16g+c, hh, t]
    dtp_all = np.ascontiguousarray(
        D.transpose(4, 0, 1, 6, 2, 5, 3).reshape(NCORES, 2 * L, C, 2, T)
    ).astype(BF)
    del D

    # cstat: [16g+c, 8b+g] = condition[b, c]
    cstat = np.zeros((C, 64), f32)
    for g in range(8):
        cstat[16 * g : 16 * g + 16, g::8] = condition.T
    cstat = cstat.astype(BF)

    # wtp per core b: [cin, (l,br,tap), cout], pre-scaled by 1/s[l,br,b]
    Wg = np.stack([np.asarray(Wt, f32), np.asarray(Ws, f32)], axis=1)
    wtp_all = []
    for b in range(B):
        Wgb = Wg * (1.0 / scl[:, :, b])[:, :, None, None, None]
        wtp_all.append(np.ascontiguousarray(
            Wgb.transpose(3, 0, 1, 2, 4).reshape(C, 6 * L, C)
        ).astype(BF))
    # wsr: [cin, (l, skip/res), cout]
    Ssr = np.stack([np.asarray(Wskip, f32)[:, 0], np.asarray(Wres, f32)[:, 0]], axis=1)
    wsr = np.ascontiguousarray(Ssr.transpose(2, 0, 1, 3).reshape(C, 2 * L, C)).astype(BF)

    w1p = np.ascontiguousarray(np.asarray(W1, f32).transpose(1, 0, 2)).astype(BF)
    b1p = np.ascontiguousarray(np.asarray(b1, f32).reshape(16, C).T)
    w2p = np.ascontiguousarray(
        np.asarray(W2, f32).reshape(3, 16, C, 2, C).transpose(2, 0, 1, 3, 4)
        .reshape(C, 96, C)
    ).astype(BF)
    b2p = np.ascontiguousarray(np.asarray(b2, f32).reshape(2, C).T)
    w3p = np.ascontiguousarray(np.asarray(W3, f32)[0, :, 0].reshape(2, C).T).astype(BF)
    b3p = np.asarray(b3, f32).reshape(1, 1)
    wcT = np.ascontiguousarray(np.asarray(Wc, f32).reshape(1, C))
    bcp = np.asarray(bc, f32).reshape(C, 1)

    base = {
        "wcT": wcT, "bcp": bcp, "cstat": cstat, "wsr": wsr,
        "w1p": w1p, "b1p": b1p, "w2p": w2p, "b2p": b2p, "w3p": w3p,
        "b3p": b3p, "invsp": invsp,
    }
    if has_p:
        P = np.stack(
            [
                np.asarray(Bt, f32) + np.asarray(bt, f32)[:, None, :],
                np.asarray(Bs, f32) + np.asarray(bs, f32)[:, None, :],
            ],
            axis=1,
        )  # [L, 2, T, C]
        P = P.reshape(L, 2, T, 8, 2, 8)
        ptp_all = np.ascontiguousarray(
            P.transpose(3, 0, 1, 4, 5, 2).reshape(NCORES, 2 * L, 16, T)
        ).astype(BF)
        del P
        pstat = np.zeros((8, 64), f32)
        for g in range(8):
            pstat[g, g::8] = 1.0
        base["pstat"] = pstat.astype(BF)
    if has_bres:
        base["bresp"] = np.ascontiguousarray(np.asarray(bres, f32).T)
    if has_bskip:
        base["bskips"] = np.asarray(bskip, f32).sum(axis=0).reshape(C, 1)

    in_maps = []
    for j in range(NCORES):
        m = dict(base)
        m["xw"] = np.ascontiguousarray(x[j, :, 0].reshape(1, T))
        m["dtp"] = dtp_all[j]
        m["wtp"] = wtp_all[j]
        m["spp"] = spp_all[j]
        if has_p:
            m["ptp"] = ptp_all[j]
        in_maps.append(m)
    return in_maps, (has_p, has_bres, has_bskip)


def kernel(**inputs) -> np.ndarray:
    in_maps, flags = _pack_inputs(**inputs)
    nc = _get_nc(*flags)
    res = bass_utils.run_bass_kernel_spmd(
        nc, in_maps, core_ids=list(range(NCORES))
    )
    outs = [res.results[j]["out"].reshape(T, 1) for j in range(NCORES)]
    return np.stack(outs, axis=0).astype(np.float32)
